# revision 44
# baseline (speedup 1.0000x reference)
"""Trainium2 Bass kernel for nn_DiffTime (embedding_lookup, 8 NeuronCores).

Computation (see reference):
    h1 = tanh(times * h1_k + h1_b)            [B, 100]
    tv = tanh(h1 @ h2_k + h2_b)               [B, 100]
    mat_x = (emb_x @ evoke_k + evoke_b)       [B, 100p, 100h]   (x in {target, context})
    mv_x = einsum('bph,bh->bp', mat_x, tv)    [B, 100]
    vect_x = mv_x @ last_k + last_b           [B, 300]
    logits = sum(vect_t * vect_c, -1)         [B]
    out = mean(softplus(logits) - logits * labels)

Strategy (data-parallel over batch, 2048 items/core, no collectives):

* tv[b,:] lies on a smooth 1-D curve of the scalar times[b]; its rank-4
  SVD basis Vr (host precompute from weights only) reproduces tv to
  ~1.3e-3, far inside the 2e-2 tolerance.  The kernel contracts emb with
  Wr[e,(p,k)] = sum_h evoke[e,p*100+h]*Vr[h,k], k = 4 coeffs + 1
  homogeneous slot, so the moving matmul dim is 505 instead of 10000.

* The per-sample coefficients c'[b,:5] = [tv(t_b)@Vr, 1] are a pure
  function of times[b]: quantize t to a 4096-level grid, precompute the
  grid on host, and fetch c' with one dma_gather (max err ~1e-4).

* The Gram matrix Gh = last_kh@last_kh.T (homogeneous coord folds
  last_b) is folded into the context branch's Wr on host, so
  logits[b] = sum_p mvt_h[b,p] * mvcg[b,p] -- one fused mul+reduce.

* Embedding rows are fetched in two gather stages: 4 sorted int16
  segment gathers (vocab split into <=32768-row spans) land rows in an
  SBUF scratch, then an SBUF-source *transpose-mode* dma_gather
  restores batch order while directly producing the [e%128, e//128, b]
  lhsT layout the PE needs -- no PE transposes, no DRAM roundtrip.

* Per 128-row chunk and branch: 3 accumulating matmuls (K=3x128,
  N=505) -> PSUM; one broadcast tensor_mul with c' (stride-0 AP) and
  one reduce over k -> mv.  Loss tail is Softplus+accum on the scalar
  engine plus one fused mul+reduce; each core returns a partial sum.
"""

import sys

for _p in ("/opt/trn_rl_repo", "/opt/trn_rl_repo/concourse"):
    if _p not in sys.path:
        sys.path.insert(0, _p)

from contextlib import ExitStack

import ml_dtypes
import numpy as np

import concourse.bacc as bacc
import concourse.bass as bass
import concourse.tile as tile
from concourse import mybir
from concourse.bass_utils import run_bass_kernel_spmd

F32 = mybir.dt.float32
F32R = mybir.dt.float32r
BF16 = mybir.dt.bfloat16
I16 = mybir.dt.int16
I32 = mybir.dt.int32
AF = mybir.ActivationFunctionType
AX = mybir.AxisListType
OP = mybir.AluOpType

N_CORES = 8
B = 16384
BC = B // N_CORES          # 2048 batch items per core
NB = BC // 128             # 16 chunks of 128 batch rows
NG = 4                     # realign gather groups
GTOK = BC // NG            # 512 tokens per realign gather
V = 100000
EMB = 300
EPAD = 384                 # padded embedding row (col 300 = 1.0, rest 0)
H = 100                    # h1 = h2 = prod dims
R = 4                      # tv-curve basis rank
RK = R + 1                 # + homogeneous slot
MH = H + 1                 # homogeneous mv size
NPR = MH * RK              # 505 contracted columns
TQ = 4096                  # time-grid levels
SEG_BASE = [0, 32768, 65536, 98304]
SEG_CAP = [768, 768, 768, 128]   # fixed (SPMD-stable) per-segment capacity
S_TOT = sum(SEG_CAP)             # 2432 scratch rows
assert S_TOT % 128 == 0


def _wrap16(v):
    """int16 index array -> dma_gather SBUF layout [128, len//16]."""
    v = np.asarray(v, dtype=np.int16)
    a = v.reshape(-1, 16).T          # [16, len/16]; slot j at [j%16, j//16]
    return np.tile(a, (8, 1))        # replicate across the 8 q7 cores


def _prep_indices(idx):
    """Sort a core's indices into int16-addressable segments.

    Returns (seg_idx [128, S_TOT//16], realign [128, BC//16]) int16 arrays.
    seg_idx holds per-segment local indices (padded with 0); realign maps
    original batch position j -> scratch row of its gathered embedding.
    """
    idx = np.asarray(idx).astype(np.int64)
    assert idx.shape == (BC,)
    order = np.argsort(idx, kind="stable")
    sidx = idx[order]
    bounds = np.searchsorted(sidx, SEG_BASE + [V])
    seg_cols = []
    counts = []
    scratch_rows = np.empty(BC, dtype=np.int64)
    off = 0
    for s in range(4):
        lo, hi = bounds[s], bounds[s + 1]
        n = hi - lo
        assert n <= SEG_CAP[s], f"segment {s} overflow: {n} > {SEG_CAP[s]}"
        local = np.full(SEG_CAP[s], -1, dtype=np.int16)
        local[:n] = sidx[lo:hi] - SEG_BASE[s]
        if n == 0:
            local[0] = 0      # keep >=1 valid idx for the q7 kernel
            n = 1
        counts.append(n)
        seg_cols.append(_wrap16(local))
        scratch_rows[lo:hi] = off + np.arange(hi - lo)
        off += SEG_CAP[s]
    realign = np.empty(BC, dtype=np.int64)
    realign[order] = scratch_rows
    return (np.hstack(seg_cols), _wrap16(realign),
            np.array(counts, dtype=np.int32).reshape(1, 4))


def _build_kernel(ctx: ExitStack, tc: "tile.TileContext", io: dict):
    nc = tc.nc

    cpool = ctx.enter_context(tc.tile_pool(name="const", bufs=1))
    wpool = ctx.enter_context(tc.tile_pool(name="work", bufs=4))
    pmm = ctx.enter_context(tc.tile_pool(name="pmm", bufs=4, space="PSUM"))

    # ---- indices + small constants -------------------------------------
    idx_sb = {}
    segcnt = {}
    for br in ("t", "c"):
        idx_sb[br] = cpool.tile([128, S_TOT // 16], I16, tag=f"idx_{br}",
                                name=f"idx_{br}")
        nc.sync.dma_start(out=idx_sb[br][:], in_=io[f"idx_{br}"][:, :])
        idx_sb[br + "r"] = cpool.tile([128, BC // 16], I16, tag=f"rel_{br}",
                                      name=f"rel_{br}")
        nc.sync.dma_start(out=idx_sb[br + "r"][:], in_=io[f"rel_{br}"][:, :])
        segcnt[br] = cpool.tile([1, 4], I32, tag=f"cnt_{br}",
                                name=f"cnt_{br}")
        nc.sync.dma_start(out=segcnt[br][:], in_=io[f"cnt_{br}"][:, :])
    w_sb = {}
    for br in ("t", "c"):
        w_sb[br] = cpool.tile([128, 3, NPR], BF16, tag=f"w_{br}",
                              name=f"w_{br}")
        nc.sync.dma_start(out=w_sb[br][:],
                          in_=io[f"w{br}"].rearrange("(j p) c -> p j c", p=128))
    labels = cpool.tile([128, NB], F32, tag="labels")
    nc.sync.dma_start(out=labels[:], in_=io["labels"][:, :])
    times = cpool.tile([1, BC], F32, tag="times")
    nc.sync.dma_start(out=times[:], in_=io["times"][:, :])
    h1k = cpool.tile([H, 1], F32, tag="h1k")
    nc.sync.dma_start(out=h1k[:], in_=io["h1k"][:, :])
    h1b = cpool.tile([H, 1], F32, tag="h1b")
    nc.sync.dma_start(out=h1b[:], in_=io["h1b"][:, :])
    h2kb = cpool.tile([H + 1, H], BF16, tag="h2kb")
    nc.sync.dma_start(out=h2kb[:], in_=io["h2kb"][:, :])
    vr = cpool.tile([H, R], F32, tag="vr")
    nc.sync.dma_start(out=vr[:], in_=io["vr"][:, :])
    ones1 = cpool.tile([1, H], F32, tag="ones1")
    nc.vector.memset(ones1[:], 1.0)

    # ---- gathers -------------------------------------------------------
    # Stage 1 (per branch): 4 sorted-segment row gathers table -> SBUF
    # scratch; scratch row j lands at partition j%128, slot j//128, which
    # is exactly the tokens_per_rank=128 layout the SBUF-source transpose
    # gather expects (token id == scratch row id).
    scratch = {}
    embT = {"t": [], "c": []}

    def seg_gathers(br, tab, q):
        scratch[br] = cpool.tile([128, S_TOT // 128, EPAD], BF16,
                                 tag=f"scr_{br}", name=f"scr_{br}")
        off = 0
        for s in range(4):
            cap = SEG_CAP[s]
            seg_len = min(32768, V - SEG_BASE[s])
            cnt = nc.gpsimd.alloc_register()
            nc.gpsimd.load(cnt, segcnt[br][0:1, s:s + 1])
            nc.gpsimd.dma_gather(
                scratch[br][:, off // 128:(off + cap) // 128, :],
                tab[SEG_BASE[s]:SEG_BASE[s] + seg_len, :],
                idx_sb[br][:, off // 16:(off + cap) // 16],
                cap, cnt, EPAD, queue_num=0, single_packet=False)
            off += cap

    def realign(br, g, ntok):
        e = cpool.tile([128, 3, ntok], BF16, tag=f"embT_{br}{g}",
                       name=f"embT_{br}{g}")
        nc.gpsimd.dma_gather(
            e[:], scratch[br][:],
            idx_sb[br + "r"][:, (ntok // 16) * g:(ntok // 16) * (g + 1)],
            ntok, ntok, EPAD, transpose=True, queue_num=0,
            single_packet=False,
            sbuf_tokens_per_rank=128, sbuf_free_dim_per_rank=EPAD * 2)
        embT[br].append(e)

    # Issue order keeps the single q7 descgen queue busy with no idle
    # holes: target realigns 0-1 let compute-t start early, context
    # segments stream their DMAs while rt2/rt3 descgen runs, and the
    # context realigns find scratch_c already landed.
    seg_gathers("t", io["ttab"], 0)
    realign("t", 0, GTOK)
    realign("t", 1, GTOK)
    seg_gathers("c", io["ctab"], 0)
    realign("t", 2, GTOK)
    realign("t", 3, GTOK)
    for g in range(NG):
        realign("c", g, GTOK)

    # ---- time MLP -> c' coefficients, batched on PE/ACT ----------------
    # h1T[j, b] = tanh(h1k[j] * t[b] + h1b[j]); tv = tanh(h2kb.T @ h1T);
    # c' chunk g = tvT_g.T @ vr, bf16, with homogeneous slot k=R set to 1.
    pmlp = ctx.enter_context(tc.tile_pool(name="pmlp", bufs=2, space="PSUM"))
    pcf = ctx.enter_context(tc.tile_pool(name="pcf", bufs=1, space="PSUM"))
    h1T = cpool.tile([H + 1, BC], BF16, tag="h1T")
    nc.vector.memset(h1T[96:H + 1, :], 1.0)   # row 100 stays 1; 96-99 overwritten
    tvT = cpool.tile([H, BC], F32, tag="tvT")
    HB = BC // 4
    for i in range(4):
        sl = slice(HB * i, HB * (i + 1))
        bc = pmlp.tile([H, HB], F32, tag="mlp", name=f"bcast{i}")
        nc.tensor.matmul(bc[:], ones1[:], times[0:1, sl], start=True,
                         stop=True)
        nc.scalar.activation(h1T[0:H, sl], bc[:], AF.Tanh, bias=h1b[:],
                             scale=h1k[:])
        tvp = pmlp.tile([H, HB], F32, tag="mlp", name=f"tvp{i}")
        nc.tensor.matmul(tvp[:], h2kb[:], h1T[:, sl], start=True, stop=True)
        nc.scalar.activation(tvT[:, sl], tvp[:], AF.Tanh)
    call = cpool.tile([128, NB, RK], BF16, tag="call")
    cfp = pcf.tile([128, NB * R], F32, tag="cfp")
    for g in range(NB):
        nc.tensor.matmul(cfp[:, R * g:R * (g + 1)],
                         tvT[:, 128 * g:128 * (g + 1)], vr[:],
                         start=True, stop=True)
        nc.scalar.copy(call[:, g, 0:R], cfp[:, R * g:R * (g + 1)])
    nc.vector.memset(call[:, :, R:RK], 1.0)

    # ---- main loop: matU = embT.T @ W ; mv = sum_k matU * c' ------------
    mv_all = {}
    junk = cpool.tile([128, NB, MH], F32, tag="junk")
    logits = cpool.tile([128, NB], F32, tag="logits")
    ab = cpool.tile([128, NB], F32, tag="ab")
    ex = cpool.tile([128, NB], F32, tag="ex")
    rl = cpool.tile([128, NB], F32, tag="rl")
    lyj = cpool.tile([128, NB], F32, tag="lyj")
    sacc = cpool.tile([128, NG + 1], F32, tag="sacc")
    for br in ("t", "c"):
        mv_all[br] = cpool.tile([128, NB, MH], F32, tag=f"mv_{br}",
                                name=f"mv_{br}")
        for c in range(NB):
            g, m = c // NG, c % NG
            matU = pmm.tile([128, NPR], F32, tag="matU", name=f"mU_{br}{c}")
            for j in range(3):
                nc.tensor.matmul(
                    matU[:], embT[br][g][:, j, 128 * m:128 * (m + 1)],
                    w_sb[br][:, j, :], start=(j == 0), stop=(j == 2))
            prod = wpool.tile([128, NPR], BF16, tag="prod")
            in0 = matU[:].rearrange("p (a k) -> p a k", k=RK)
            in1 = call[:, c:c + 1, 0:RK]
            b0, b1 = bass.broadcast_tensor_aps(in0, in1)
            nc.vector.tensor_tensor(
                prod[:].rearrange("p (a k) -> p a k", k=RK), b0, b1,
                op=OP.mult)
            nc.vector.reduce_sum(
                out=mv_all[br][:, c, :],
                in_=prod[:].rearrange("p (a k) -> p a k", k=RK), axis=AX.X)
            if br == "c" and c % NG == NG - 1:
                # logits + softplus pieces for this group while gathers
                # stream; Abs/Exp/Relu share the tanh act table (no swap),
                # the lone Ln runs batched at the very end.
                g4 = c // NG
                gs = slice(c - NG + 1, c + 1)
                nc.vector.tensor_mul(junk[:, gs, :], mv_all["t"][:, gs, :],
                                     mv_all["c"][:, gs, :])
                nc.vector.reduce_sum(out=logits[:, gs], in_=junk[:, gs, :],
                                     axis=AX.X)
                nc.scalar.activation(ab[:, gs], logits[:, gs], AF.Abs)
                nc.scalar.activation(ex[:, gs], ab[:, gs], AF.Exp, scale=-1.0)
                nc.scalar.activation(rl[:, gs], logits[:, gs], AF.Relu,
                                     accum_out=sacc[:, g4:g4 + 1])
                nc.vector.tensor_mul(lyj[:, gs], logits[:, gs], labels[:, gs])

    # ---- final reduction ------------------------------------------------
    l1p = cpool.tile([128, NB], F32, tag="l1p")
    nc.scalar.activation(l1p[:], ex[:], AF.Ln, bias=1.0,
                         accum_out=sacc[:, NG:NG + 1])
    spos = cpool.tile([128, 1], F32, tag="spos")
    nc.vector.reduce_sum(out=spos[:], in_=sacc[:], axis=AX.X)
    s2 = cpool.tile([128, 1], F32, tag="s2")
    nc.vector.reduce_sum(out=s2[:], in_=lyj[:], axis=AX.X)
    srow = cpool.tile([128, 1], F32, tag="srow")
    nc.vector.tensor_sub(srow[:], spos[:], s2[:])
    nc.sync.dma_start(out=io["out"][:, :], in_=srow[:])


_PROGRAM = None


def _get_program():
    global _PROGRAM
    if _PROGRAM is not None:
        return _PROGRAM
    nc = bacc.Bacc("TRN2", target_bir_lowering=False, debug=False,
                   num_devices=N_CORES)
    io = {
        "ttab": nc.dram_tensor("ttab", [V, EPAD], BF16, kind="ExternalInput").ap(),
        "ctab": nc.dram_tensor("ctab", [V, EPAD], BF16, kind="ExternalInput").ap(),
        "wt": nc.dram_tensor("wt", [EPAD, NPR], BF16, kind="ExternalInput").ap(),
        "wc": nc.dram_tensor("wc", [EPAD, NPR], BF16, kind="ExternalInput").ap(),
        "times": nc.dram_tensor("times", [1, BC], F32, kind="ExternalInput").ap(),
        "h1k": nc.dram_tensor("h1k", [H, 1], F32, kind="ExternalInput").ap(),
        "h1b": nc.dram_tensor("h1b", [H, 1], F32, kind="ExternalInput").ap(),
        "h2kb": nc.dram_tensor("h2kb", [H + 1, H], BF16, kind="ExternalInput").ap(),
        "vr": nc.dram_tensor("vr", [H, R], F32, kind="ExternalInput").ap(),
        "labels": nc.dram_tensor("labels", [128, NB], F32, kind="ExternalInput").ap(),
        "idx_t": nc.dram_tensor("idx_t", [128, S_TOT // 16], I16, kind="ExternalInput").ap(),
        "idx_c": nc.dram_tensor("idx_c", [128, S_TOT // 16], I16, kind="ExternalInput").ap(),
        "rel_t": nc.dram_tensor("rel_t", [128, BC // 16], I16, kind="ExternalInput").ap(),
        "rel_c": nc.dram_tensor("rel_c", [128, BC // 16], I16, kind="ExternalInput").ap(),
        "cnt_t": nc.dram_tensor("cnt_t", [1, 4], I32, kind="ExternalInput").ap(),
        "cnt_c": nc.dram_tensor("cnt_c", [1, 4], I32, kind="ExternalInput").ap(),
        "out": nc.dram_tensor("out", [128, 1], F32, kind="ExternalOutput").ap(),
    }
    with tile.TileContext(nc) as tc:
        with ExitStack() as ctx:
            _build_kernel(ctx, tc, io)
    nc.compile()
    _PROGRAM = nc
    return nc


def _pad_table(tab):
    out = np.zeros((V, EPAD), dtype=ml_dtypes.bfloat16)
    out[:, :EMB] = np.asarray(tab).astype(ml_dtypes.bfloat16)
    out[:, EMB] = 1.0
    return out


def _tv_curve(h1_k, h1_b, h2_k, h2_b, t):
    h1 = np.tanh(t.reshape(-1, 1) @ np.asarray(h1_k, np.float64).reshape(1, H)
                 + np.asarray(h1_b, np.float64).reshape(H))
    return np.tanh(h1 @ np.asarray(h2_k, np.float64)
                   + np.asarray(h2_b, np.float64).reshape(H))


def _tv_basis(h1_k, h1_b, h2_k, h2_b):
    """Top-R right singular basis of the tv curve (weights-only precompute)."""
    g = np.linspace(0.0, 1.0, 2049, dtype=np.float64)
    tvg = _tv_curve(h1_k, h1_b, h2_k, h2_b, g)
    _, _, vt = np.linalg.svd(tvg, full_matrices=False)
    return np.ascontiguousarray(vt[:R].T)          # [100, R]


def build_in_maps(targets, contexts, times, labels, targetemb, contextemb,
                  h1_k, h1_b, h2_k, h2_b, evoke_k, evoke_b, last_k, last_b):
    ttab = _pad_table(targetemb)
    ctab = _pad_table(contextemb)
    vrb = _tv_basis(h1_k, h1_b, h2_k, h2_b)        # [100, R] float64
    h1kc = np.asarray(h1_k, np.float32).reshape(1, H).T.copy()
    h1bc = np.asarray(h1_b, np.float32).reshape(H, 1).copy()
    h2kb = np.vstack([np.asarray(h2_k), np.asarray(h2_b).reshape(1, H)]
                     ).astype(ml_dtypes.bfloat16)

    # Wr[e, p, k] = sum_h evoke_pad[e, p*100+h] * Vr[h, k]
    evoke_pad = np.zeros((EPAD, H * H), dtype=np.float64)
    evoke_pad[:EMB, :] = np.asarray(evoke_k, np.float64)
    evoke_pad[EMB, :] = np.asarray(evoke_b, np.float64)
    wr4 = (evoke_pad.reshape(EPAD * H, H) @ vrb).reshape(EPAD, H, R)

    # target branch: homogeneous slot (p=100, k=R) = 1
    wt = np.zeros((EPAD, MH, RK), dtype=np.float64)
    wt[:, :H, :R] = wr4
    wt[EMB, H, R] = 1.0

    # context branch: fold Gh = last_kh @ last_kh.T
    lastkh = np.vstack([np.asarray(last_k, np.float64),
                        np.asarray(last_b, np.float64).reshape(1, EMB)])
    gh = lastkh @ lastkh.T                         # [101, 101]
    wc = np.zeros((EPAD, MH, RK), dtype=np.float64)
    wc[:, :, :R] = np.einsum("pq,eqk->epk", gh[:, :H], wr4)
    wc[EMB, :, R] = gh[:, H]

    wt = wt.reshape(EPAD, NPR).astype(ml_dtypes.bfloat16)
    wc = wc.reshape(EPAD, NPR).astype(ml_dtypes.bfloat16)

    targets = np.asarray(targets)
    contexts = np.asarray(contexts)
    times = np.asarray(times).astype(np.float32)
    labels = np.asarray(labels).astype(np.float32)

    in_maps = []
    for k in range(N_CORES):
        sl = slice(k * BC, (k + 1) * BC)
        idx_t, rel_t, cnt_t = _prep_indices(targets[sl])
        idx_c, rel_c, cnt_c = _prep_indices(contexts[sl])
        in_maps.append({
            "ttab": ttab, "ctab": ctab, "wt": wt, "wc": wc,
            "times": times[sl].reshape(1, BC),
            "h1k": h1kc, "h1b": h1bc, "h2kb": h2kb,
            "vr": vrb.astype(np.float32),
            "labels": labels[sl].reshape(NB, 128).T.copy(),
            "idx_t": idx_t, "idx_c": idx_c, "rel_t": rel_t, "rel_c": rel_c,
            "cnt_t": cnt_t, "cnt_c": cnt_c,
        })
    return in_maps


def kernel(**inputs) -> np.ndarray:
    nc = _get_program()
    in_maps = build_in_maps(**inputs)
    r = run_bass_kernel_spmd(nc, in_maps, list(range(N_CORES)))
    total = np.float64(0.0)
    for m in r.results:
        total += np.asarray(m["out"], np.float64).sum()
    return np.float32(total / B)


# revision 45
# speedup vs baseline: 1.1065x; 1.1065x over previous
"""Trainium2 Bass kernel for nn_DiffTime (embedding_lookup, 8 NeuronCores).

Computation (see reference):
    h1 = tanh(times * h1_k + h1_b)            [B, 100]
    tv = tanh(h1 @ h2_k + h2_b)               [B, 100]
    mat_x = (emb_x @ evoke_k + evoke_b)       [B, 100p, 100h]   (x in {target, context})
    mv_x = einsum('bph,bh->bp', mat_x, tv)    [B, 100]
    vect_x = mv_x @ last_k + last_b           [B, 300]
    logits = sum(vect_t * vect_c, -1)         [B]
    out = mean(softplus(logits) - logits * labels)

Strategy (data-parallel over batch, 2048 items/core, no collectives):

* tv[b,:] lies on a smooth 1-D curve of the scalar times[b]; its rank-4
  SVD basis Vr (host precompute from weights only) reproduces tv to
  ~1.3e-3, far inside the 2e-2 tolerance.  The kernel contracts emb with
  Wr[e,(p,k)] = sum_h evoke[e,p*100+h]*Vr[h,k], k = 4 coeffs + 1
  homogeneous slot, so the moving matmul dim is 505 instead of 10000.

* The per-sample coefficients c'[b,:5] = [tv(t_b)@Vr, 1] are a pure
  function of times[b]: quantize t to a 4096-level grid, precompute the
  grid on host, and fetch c' with one dma_gather (max err ~1e-4).

* The Gram matrix Gh = last_kh@last_kh.T (homogeneous coord folds
  last_b) is folded into the context branch's Wr on host, so
  logits[b] = sum_p mvt_h[b,p] * mvcg[b,p] -- one fused mul+reduce.

* Embedding rows are fetched in two gather stages: 4 sorted int16
  segment gathers (vocab split into <=32768-row spans) land rows in an
  SBUF scratch, then an SBUF-source *transpose-mode* dma_gather
  restores batch order while directly producing the [e%128, e//128, b]
  lhsT layout the PE needs -- no PE transposes, no DRAM roundtrip.

* Per 128-row chunk and branch: 3 accumulating matmuls (K=3x128,
  N=505) -> PSUM; one broadcast tensor_mul with c' (stride-0 AP) and
  one reduce over k -> mv.  Loss tail is Softplus+accum on the scalar
  engine plus one fused mul+reduce; each core returns a partial sum.
"""

import sys

for _p in ("/opt/trn_rl_repo", "/opt/trn_rl_repo/concourse"):
    if _p not in sys.path:
        sys.path.insert(0, _p)

from contextlib import ExitStack

import ml_dtypes
import numpy as np

import concourse.bacc as bacc
import concourse.bass as bass
import concourse.tile as tile
from concourse import mybir
from concourse.bass_utils import run_bass_kernel_spmd

F32 = mybir.dt.float32
F32R = mybir.dt.float32r
BF16 = mybir.dt.bfloat16
I16 = mybir.dt.int16
I32 = mybir.dt.int32
AF = mybir.ActivationFunctionType
AX = mybir.AxisListType
OP = mybir.AluOpType

N_CORES = 8
B = 16384
BC = B // N_CORES          # 2048 batch items per core
NB = BC // 128             # 16 chunks of 128 batch rows
NG = 4                     # realign gather groups
GTOK = BC // NG            # 512 tokens per realign gather
V = 100000
EMB = 300
EPAD = 384                 # padded embedding row (col 300 = 1.0, rest 0)
H = 100                    # h1 = h2 = prod dims
R = 4                      # tv-curve basis rank
RK = R + 1                 # + homogeneous slot
MH = H + 1                 # homogeneous mv size
NPR = MH * RK              # 505 contracted columns
TQ = 4096                  # time-grid levels
SEG_BASE = [0, 32768, 65536, 98304]
SEG_CAP = [768, 768, 768, 128]   # fixed (SPMD-stable) per-segment capacity
S_TOT = sum(SEG_CAP)             # 2432 scratch rows
assert S_TOT % 128 == 0


def _wrap16(v):
    """int16 index array -> dma_gather SBUF layout [128, len//16]."""
    v = np.asarray(v, dtype=np.int16)
    a = v.reshape(-1, 16).T          # [16, len/16]; slot j at [j%16, j//16]
    return np.tile(a, (8, 1))        # replicate across the 8 q7 cores


def _prep_indices(idx):
    """Sort a core's indices into int16-addressable segments.

    Returns (seg_idx [128, S_TOT//16], realign [128, BC//16]) int16 arrays.
    seg_idx holds per-segment local indices (padded with 0); realign maps
    original batch position j -> scratch row of its gathered embedding.
    """
    idx = np.asarray(idx).astype(np.int64)
    assert idx.shape == (BC,)
    order = np.argsort(idx, kind="stable")
    sidx = idx[order]
    bounds = np.searchsorted(sidx, SEG_BASE + [V])
    seg_cols = []
    scratch_rows = np.empty(BC, dtype=np.int64)
    off = 0
    for s in range(4):
        lo, hi = bounds[s], bounds[s + 1]
        n = hi - lo
        assert n <= SEG_CAP[s], f"segment {s} overflow: {n} > {SEG_CAP[s]}"
        local = np.zeros(SEG_CAP[s], dtype=np.int16)
        local[:n] = sidx[lo:hi] - SEG_BASE[s]
        seg_cols.append(_wrap16(local))
        scratch_rows[lo:hi] = off + np.arange(n)
        off += SEG_CAP[s]
    realign = np.empty(BC, dtype=np.int64)
    realign[order] = scratch_rows
    return np.hstack(seg_cols), _wrap16(realign)


def _build_kernel(ctx: ExitStack, tc: "tile.TileContext", io: dict):
    nc = tc.nc

    cpool = ctx.enter_context(tc.tile_pool(name="const", bufs=1))
    wpool = ctx.enter_context(tc.tile_pool(name="work", bufs=4))
    pmm = ctx.enter_context(tc.tile_pool(name="pmm", bufs=4, space="PSUM"))

    # ---- indices + small constants -------------------------------------
    idx_sb = {}
    for br in ("t", "c"):
        idx_sb[br] = cpool.tile([128, S_TOT // 16], I16, tag=f"idx_{br}",
                                name=f"idx_{br}")
        nc.sync.dma_start(out=idx_sb[br][:], in_=io[f"idx_{br}"][:, :])
        idx_sb[br + "r"] = cpool.tile([128, BC // 16], I16, tag=f"rel_{br}",
                                      name=f"rel_{br}")
        nc.sync.dma_start(out=idx_sb[br + "r"][:], in_=io[f"rel_{br}"][:, :])
    w_sb = {}
    for br in ("t", "c"):
        w_sb[br] = cpool.tile([128, 3, NPR], BF16, tag=f"w_{br}",
                              name=f"w_{br}")
        nc.sync.dma_start(out=w_sb[br][:],
                          in_=io[f"w{br}"].rearrange("(j p) c -> p j c", p=128))
    labels = cpool.tile([128, NB], F32, tag="labels")
    nc.sync.dma_start(out=labels[:], in_=io["labels"][:, :])
    times = cpool.tile([1, BC], F32, tag="times")
    nc.sync.dma_start(out=times[:], in_=io["times"][:, :])
    h1k = cpool.tile([H, 1], F32, tag="h1k")
    nc.sync.dma_start(out=h1k[:], in_=io["h1k"][:, :])
    h1b = cpool.tile([H, 1], F32, tag="h1b")
    nc.sync.dma_start(out=h1b[:], in_=io["h1b"][:, :])
    h2kb = cpool.tile([H + 1, H], BF16, tag="h2kb")
    nc.sync.dma_start(out=h2kb[:], in_=io["h2kb"][:, :])
    vr = cpool.tile([H, R], F32, tag="vr")
    nc.sync.dma_start(out=vr[:], in_=io["vr"][:, :])
    ones1 = cpool.tile([1, H], F32, tag="ones1")
    nc.vector.memset(ones1[:], 1.0)

    # ---- gathers -------------------------------------------------------
    # Stage 1 (per branch): 4 sorted-segment row gathers table -> SBUF
    # scratch; scratch row j lands at partition j%128, slot j//128, which
    # is exactly the tokens_per_rank=128 layout the SBUF-source transpose
    # gather expects (token id == scratch row id).
    scratch = {}
    embT = {"t": [], "c": []}

    def seg_gathers(br, tab, q):
        scratch[br] = cpool.tile([128, S_TOT // 128, EPAD], BF16,
                                 tag=f"scr_{br}", name=f"scr_{br}")
        off = 0
        for s in range(4):
            cap = SEG_CAP[s]
            seg_len = min(32768, V - SEG_BASE[s])
            nc.gpsimd.dma_gather(
                scratch[br][:, off // 128:(off + cap) // 128, :],
                tab[SEG_BASE[s]:SEG_BASE[s] + seg_len, :],
                idx_sb[br][:, off // 16:(off + cap) // 16],
                cap, cap, EPAD, queue_num=0, single_packet=False)
            off += cap

    def realign(br, g, ntok):
        e = cpool.tile([128, 3, ntok], BF16, tag=f"embT_{br}{g}",
                       name=f"embT_{br}{g}")
        nc.gpsimd.dma_gather(
            e[:], scratch[br][:],
            idx_sb[br + "r"][:, (ntok // 16) * g:(ntok // 16) * (g + 1)],
            ntok, ntok, EPAD, transpose=True, queue_num=0,
            single_packet=False,
            sbuf_tokens_per_rank=128, sbuf_free_dim_per_rank=EPAD * 2)
        embT[br].append(e)

    # Issue order keeps the single q7 descgen queue busy with no idle
    # holes: target realigns 0-1 let compute-t start early, context
    # segments stream their DMAs while rt2/rt3 descgen runs, and the
    # context realigns find scratch_c already landed.
    seg_gathers("t", io["ttab"], 0)
    realign("t", 0, GTOK)
    realign("t", 1, GTOK)
    seg_gathers("c", io["ctab"], 0)
    realign("t", 2, GTOK)
    realign("t", 3, GTOK)
    for g in range(NG):
        realign("c", g, GTOK)

    # ---- time MLP -> c' coefficients, batched on PE/ACT ----------------
    # h1T[j, b] = tanh(h1k[j] * t[b] + h1b[j]); tv = tanh(h2kb.T @ h1T);
    # c' chunk g = tvT_g.T @ vr, bf16, with homogeneous slot k=R set to 1.
    pmlp = ctx.enter_context(tc.tile_pool(name="pmlp", bufs=2, space="PSUM"))
    pcf = ctx.enter_context(tc.tile_pool(name="pcf", bufs=1, space="PSUM"))
    h1T = cpool.tile([H + 1, BC], BF16, tag="h1T")
    nc.vector.memset(h1T[96:H + 1, :], 1.0)   # row 100 stays 1; 96-99 overwritten
    tvT = cpool.tile([H, BC], F32, tag="tvT")
    HB = BC // 4
    for i in range(4):
        sl = slice(HB * i, HB * (i + 1))
        bc = pmlp.tile([H, HB], F32, tag="mlp", name=f"bcast{i}")
        nc.tensor.matmul(bc[:], ones1[:], times[0:1, sl], start=True,
                         stop=True)
        nc.scalar.activation(h1T[0:H, sl], bc[:], AF.Tanh, bias=h1b[:],
                             scale=h1k[:])
        tvp = pmlp.tile([H, HB], F32, tag="mlp", name=f"tvp{i}")
        nc.tensor.matmul(tvp[:], h2kb[:], h1T[:, sl], start=True, stop=True)
        nc.scalar.activation(tvT[:, sl], tvp[:], AF.Tanh)
    call = cpool.tile([128, NB, RK], BF16, tag="call")
    cfp = pcf.tile([128, NB * R], F32, tag="cfp")
    for g in range(NB):
        nc.tensor.matmul(cfp[:, R * g:R * (g + 1)],
                         tvT[:, 128 * g:128 * (g + 1)], vr[:],
                         start=True, stop=True)
        nc.scalar.copy(call[:, g, 0:R], cfp[:, R * g:R * (g + 1)])
    nc.vector.memset(call[:, :, R:RK], 1.0)

    # ---- main loop: matU = embT.T @ W ; mv = sum_k matU * c' ------------
    mv_all = {}
    junk = cpool.tile([128, NB, MH], F32, tag="junk")
    logits = cpool.tile([128, NB], F32, tag="logits")
    ab = cpool.tile([128, NB], F32, tag="ab")
    ex = cpool.tile([128, NB], F32, tag="ex")
    rl = cpool.tile([128, NB], F32, tag="rl")
    lyj = cpool.tile([128, NB], F32, tag="lyj")
    sacc = cpool.tile([128, NG + 1], F32, tag="sacc")
    for br in ("t", "c"):
        mv_all[br] = cpool.tile([128, NB, MH], F32, tag=f"mv_{br}",
                                name=f"mv_{br}")
        for c in range(NB):
            g, m = c // NG, c % NG
            matU = pmm.tile([128, NPR], F32, tag="matU", name=f"mU_{br}{c}")
            for j in range(3):
                nc.tensor.matmul(
                    matU[:], embT[br][g][:, j, 128 * m:128 * (m + 1)],
                    w_sb[br][:, j, :], start=(j == 0), stop=(j == 2))
            prod = wpool.tile([128, NPR], BF16, tag="prod")
            in0 = matU[:].rearrange("p (a k) -> p a k", k=RK)
            in1 = call[:, c:c + 1, 0:RK]
            b0, b1 = bass.broadcast_tensor_aps(in0, in1)
            nc.vector.tensor_tensor(
                prod[:].rearrange("p (a k) -> p a k", k=RK), b0, b1,
                op=OP.mult)
            nc.vector.reduce_sum(
                out=mv_all[br][:, c, :],
                in_=prod[:].rearrange("p (a k) -> p a k", k=RK), axis=AX.X)
            if br == "c" and c % NG == NG - 1:
                # logits + softplus pieces for this group while gathers
                # stream; Abs/Exp/Relu share the tanh act table (no swap),
                # the lone Ln runs batched at the very end.
                g4 = c // NG
                gs = slice(c - NG + 1, c + 1)
                nc.vector.tensor_mul(junk[:, gs, :], mv_all["t"][:, gs, :],
                                     mv_all["c"][:, gs, :])
                nc.vector.reduce_sum(out=logits[:, gs], in_=junk[:, gs, :],
                                     axis=AX.X)
                nc.scalar.activation(ab[:, gs], logits[:, gs], AF.Abs)
                nc.scalar.activation(ex[:, gs], ab[:, gs], AF.Exp, scale=-1.0)
                nc.scalar.activation(rl[:, gs], logits[:, gs], AF.Relu,
                                     accum_out=sacc[:, g4:g4 + 1])
                nc.vector.tensor_mul(lyj[:, gs], logits[:, gs], labels[:, gs])

    # ---- final reduction ------------------------------------------------
    l1p = cpool.tile([128, NB], F32, tag="l1p")
    nc.scalar.activation(l1p[:], ex[:], AF.Ln, bias=1.0,
                         accum_out=sacc[:, NG:NG + 1])
    spos = cpool.tile([128, 1], F32, tag="spos")
    nc.vector.reduce_sum(out=spos[:], in_=sacc[:], axis=AX.X)
    s2 = cpool.tile([128, 1], F32, tag="s2")
    nc.vector.reduce_sum(out=s2[:], in_=lyj[:], axis=AX.X)
    srow = cpool.tile([128, 1], F32, tag="srow")
    nc.vector.tensor_sub(srow[:], spos[:], s2[:])
    nc.sync.dma_start(out=io["out"][:, :], in_=srow[:])


_PROGRAM = None


def _get_program():
    global _PROGRAM
    if _PROGRAM is not None:
        return _PROGRAM
    nc = bacc.Bacc("TRN2", target_bir_lowering=False, debug=False,
                   num_devices=N_CORES)
    io = {
        "ttab": nc.dram_tensor("ttab", [V, EPAD], BF16, kind="ExternalInput").ap(),
        "ctab": nc.dram_tensor("ctab", [V, EPAD], BF16, kind="ExternalInput").ap(),
        "wt": nc.dram_tensor("wt", [EPAD, NPR], BF16, kind="ExternalInput").ap(),
        "wc": nc.dram_tensor("wc", [EPAD, NPR], BF16, kind="ExternalInput").ap(),
        "times": nc.dram_tensor("times", [1, BC], F32, kind="ExternalInput").ap(),
        "h1k": nc.dram_tensor("h1k", [H, 1], F32, kind="ExternalInput").ap(),
        "h1b": nc.dram_tensor("h1b", [H, 1], F32, kind="ExternalInput").ap(),
        "h2kb": nc.dram_tensor("h2kb", [H + 1, H], BF16, kind="ExternalInput").ap(),
        "vr": nc.dram_tensor("vr", [H, R], F32, kind="ExternalInput").ap(),
        "labels": nc.dram_tensor("labels", [128, NB], F32, kind="ExternalInput").ap(),
        "idx_t": nc.dram_tensor("idx_t", [128, S_TOT // 16], I16, kind="ExternalInput").ap(),
        "idx_c": nc.dram_tensor("idx_c", [128, S_TOT // 16], I16, kind="ExternalInput").ap(),
        "rel_t": nc.dram_tensor("rel_t", [128, BC // 16], I16, kind="ExternalInput").ap(),
        "rel_c": nc.dram_tensor("rel_c", [128, BC // 16], I16, kind="ExternalInput").ap(),
        "out": nc.dram_tensor("out", [128, 1], F32, kind="ExternalOutput").ap(),
    }
    with tile.TileContext(nc) as tc:
        with ExitStack() as ctx:
            _build_kernel(ctx, tc, io)
    nc.compile()
    _PROGRAM = nc
    return nc


def _pad_table(tab):
    out = np.zeros((V, EPAD), dtype=ml_dtypes.bfloat16)
    out[:, :EMB] = np.asarray(tab).astype(ml_dtypes.bfloat16)
    out[:, EMB] = 1.0
    return out


def _tv_curve(h1_k, h1_b, h2_k, h2_b, t):
    h1 = np.tanh(t.reshape(-1, 1) @ np.asarray(h1_k, np.float64).reshape(1, H)
                 + np.asarray(h1_b, np.float64).reshape(H))
    return np.tanh(h1 @ np.asarray(h2_k, np.float64)
                   + np.asarray(h2_b, np.float64).reshape(H))


def _tv_basis(h1_k, h1_b, h2_k, h2_b):
    """Top-R right singular basis of the tv curve (weights-only precompute)."""
    g = np.linspace(0.0, 1.0, 2049, dtype=np.float64)
    tvg = _tv_curve(h1_k, h1_b, h2_k, h2_b, g)
    _, _, vt = np.linalg.svd(tvg, full_matrices=False)
    return np.ascontiguousarray(vt[:R].T)          # [100, R]


def build_in_maps(targets, contexts, times, labels, targetemb, contextemb,
                  h1_k, h1_b, h2_k, h2_b, evoke_k, evoke_b, last_k, last_b):
    ttab = _pad_table(targetemb)
    ctab = _pad_table(contextemb)
    vrb = _tv_basis(h1_k, h1_b, h2_k, h2_b)        # [100, R] float64
    h1kc = np.asarray(h1_k, np.float32).reshape(1, H).T.copy()
    h1bc = np.asarray(h1_b, np.float32).reshape(H, 1).copy()
    h2kb = np.vstack([np.asarray(h2_k), np.asarray(h2_b).reshape(1, H)]
                     ).astype(ml_dtypes.bfloat16)

    # Wr[e, p, k] = sum_h evoke_pad[e, p*100+h] * Vr[h, k]
    evoke_pad = np.zeros((EPAD, H * H), dtype=np.float64)
    evoke_pad[:EMB, :] = np.asarray(evoke_k, np.float64)
    evoke_pad[EMB, :] = np.asarray(evoke_b, np.float64)
    wr4 = (evoke_pad.reshape(EPAD * H, H) @ vrb).reshape(EPAD, H, R)

    # target branch: homogeneous slot (p=100, k=R) = 1
    wt = np.zeros((EPAD, MH, RK), dtype=np.float64)
    wt[:, :H, :R] = wr4
    wt[EMB, H, R] = 1.0

    # context branch: fold Gh = last_kh @ last_kh.T
    lastkh = np.vstack([np.asarray(last_k, np.float64),
                        np.asarray(last_b, np.float64).reshape(1, EMB)])
    gh = lastkh @ lastkh.T                         # [101, 101]
    wc = np.zeros((EPAD, MH, RK), dtype=np.float64)
    wc[:, :, :R] = np.einsum("pq,eqk->epk", gh[:, :H], wr4)
    wc[EMB, :, R] = gh[:, H]

    wt = wt.reshape(EPAD, NPR).astype(ml_dtypes.bfloat16)
    wc = wc.reshape(EPAD, NPR).astype(ml_dtypes.bfloat16)

    targets = np.asarray(targets)
    contexts = np.asarray(contexts)
    times = np.asarray(times).astype(np.float32)
    labels = np.asarray(labels).astype(np.float32)

    in_maps = []
    for k in range(N_CORES):
        sl = slice(k * BC, (k + 1) * BC)
        idx_t, rel_t = _prep_indices(targets[sl])
        idx_c, rel_c = _prep_indices(contexts[sl])
        in_maps.append({
            "ttab": ttab, "ctab": ctab, "wt": wt, "wc": wc,
            "times": times[sl].reshape(1, BC),
            "h1k": h1kc, "h1b": h1bc, "h2kb": h2kb,
            "vr": vrb.astype(np.float32),
            "labels": labels[sl].reshape(NB, 128).T.copy(),
            "idx_t": idx_t, "idx_c": idx_c, "rel_t": rel_t, "rel_c": rel_c,
        })
    return in_maps


def kernel(**inputs) -> np.ndarray:
    nc = _get_program()
    in_maps = build_in_maps(**inputs)
    r = run_bass_kernel_spmd(nc, in_maps, list(range(N_CORES)))
    total = np.float64(0.0)
    for m in r.results:
        total += np.asarray(m["out"], np.float64).sum()
    return np.float32(total / B)


# revision 47
# speedup vs baseline: 1.1124x; 1.0053x over previous
"""Trainium2 Bass kernel for nn_DiffTime (embedding_lookup, 8 NeuronCores).

Computation (see reference):
    h1 = tanh(times * h1_k + h1_b)            [B, 100]
    tv = tanh(h1 @ h2_k + h2_b)               [B, 100]
    mat_x = (emb_x @ evoke_k + evoke_b)       [B, 100p, 100h]   (x in {target, context})
    mv_x = einsum('bph,bh->bp', mat_x, tv)    [B, 100]
    vect_x = mv_x @ last_k + last_b           [B, 300]
    logits = sum(vect_t * vect_c, -1)         [B]
    out = mean(softplus(logits) - logits * labels)

Strategy (data-parallel over batch, 2048 items/core, no collectives):

* tv[b,:] lies on a smooth 1-D curve of the scalar times[b]; its rank-4
  SVD basis Vr (host precompute from weights only) reproduces tv to
  ~1.3e-3, far inside the 2e-2 tolerance.  The kernel contracts emb with
  Wr[e,(p,k)] = sum_h evoke[e,p*100+h]*Vr[h,k], k = 4 coeffs + 1
  homogeneous slot, so the moving matmul dim is 505 instead of 10000.

* The per-sample coefficients c'[b,:5] = [tv(t_b)@Vr, 1] come from the
  batched two-layer time MLP on PE/ACT (runs inside the gather shadow).

* The Gram matrix Gh = last_kh@last_kh.T (homogeneous coord folds
  last_b) is folded into the context branch's Wr on host, so
  logits[b] = sum_p mvt_h[b,p] * mvcg[b,p] -- one fused mul+reduce.

* Embedding rows are fetched in two gather stages: 4 sorted int16
  segment gathers (vocab split into <=32768-row spans) land rows in an
  SBUF scratch, then SBUF-source *transpose-mode* dma_gathers restore
  batch order while directly producing the [e%128, e//128, b] lhsT
  layout the PE needs -- no PE transposes, no DRAM roundtrip.  Issue
  order interleaves the two branches so the single q7 descgen queue
  (the critical resource, ~8ns/index) never idles and compute starts
  on the target branch while context gathers stream.

* Per 128-row chunk and branch: 3 accumulating matmuls (K=3x128,
  N=505) -> PSUM; one broadcast tensor_mul with c' (stride-0 AP view)
  and one reduce over k -> mv.  Logits + softplus pieces run per
  4-chunk group (Abs/Exp/Relu share one act table; the lone Ln runs
  once at the end).  Each core returns 128 row partial sums; the host
  adds them.
"""

import sys

for _p in ("/opt/trn_rl_repo", "/opt/trn_rl_repo/concourse"):
    if _p not in sys.path:
        sys.path.insert(0, _p)

from contextlib import ExitStack

import ml_dtypes
import numpy as np

import concourse.bacc as bacc
import concourse.bass as bass
import concourse.tile as tile
from concourse import mybir
from concourse.bass_utils import run_bass_kernel_spmd

F32 = mybir.dt.float32
F32R = mybir.dt.float32r
BF16 = mybir.dt.bfloat16
I16 = mybir.dt.int16
I32 = mybir.dt.int32
AF = mybir.ActivationFunctionType
AX = mybir.AxisListType
OP = mybir.AluOpType

N_CORES = 8
B = 16384
BC = B // N_CORES          # 2048 batch items per core
NB = BC // 128             # 16 chunks of 128 batch rows
NG = 4                     # realign gather groups
GTOK = BC // NG            # 512 tokens per realign gather
V = 100000
EMB = 300
EPAD = 384                 # padded embedding row (col 300 = 1.0, rest 0)
H = 100                    # h1 = h2 = prod dims
R = 4                      # tv-curve basis rank
RK = R + 1                 # + homogeneous slot
MH = H + 1                 # homogeneous mv size
NPR = MH * RK              # 505 contracted columns
TQ = 4096                  # time-grid levels
SEG_BASE = [0, 32768, 65536, 98304]
SEG_CAP = [768, 768, 768, 128]   # fixed (SPMD-stable) per-segment capacity
S_TOT = sum(SEG_CAP)             # 2432 scratch rows
assert S_TOT % 128 == 0


def _wrap16(v):
    """int16 index array -> dma_gather SBUF layout [128, len//16]."""
    v = np.asarray(v, dtype=np.int16)
    a = v.reshape(-1, 16).T          # [16, len/16]; slot j at [j%16, j//16]
    return np.tile(a, (8, 1))        # replicate across the 8 q7 cores


def _prep_indices(idx):
    """Sort a core's indices into int16-addressable segments.

    Returns (seg_idx [128, S_TOT//16], realign [128, BC//16]) int16 arrays.
    seg_idx holds per-segment local indices (padded with 0); realign maps
    original batch position j -> scratch row of its gathered embedding.
    """
    idx = np.asarray(idx).astype(np.int64)
    assert idx.shape == (BC,)
    order = np.argsort(idx, kind="stable")
    sidx = idx[order]
    bounds = np.searchsorted(sidx, SEG_BASE + [V])
    seg_cols = []
    scratch_rows = np.empty(BC, dtype=np.int64)
    off = 0
    for s in range(4):
        lo, hi = bounds[s], bounds[s + 1]
        n = hi - lo
        assert n <= SEG_CAP[s], f"segment {s} overflow: {n} > {SEG_CAP[s]}"
        local = np.zeros(SEG_CAP[s], dtype=np.int16)
        local[:n] = sidx[lo:hi] - SEG_BASE[s]
        seg_cols.append(_wrap16(local))
        scratch_rows[lo:hi] = off + np.arange(n)
        off += SEG_CAP[s]
    realign = np.empty(BC, dtype=np.int64)
    realign[order] = scratch_rows
    return np.hstack(seg_cols), _wrap16(realign)


def _build_kernel(ctx: ExitStack, tc: "tile.TileContext", io: dict):
    nc = tc.nc

    cpool = ctx.enter_context(tc.tile_pool(name="const", bufs=1))
    wpool = ctx.enter_context(tc.tile_pool(name="work", bufs=4))
    pmm = ctx.enter_context(tc.tile_pool(name="pmm", bufs=2, space="PSUM"))

    # ---- indices + small constants -------------------------------------
    idx_sb = {}
    for br in ("t", "c"):
        idx_sb[br] = cpool.tile([128, S_TOT // 16], I16, tag=f"idx_{br}",
                                name=f"idx_{br}")
        nc.sync.dma_start(out=idx_sb[br][:], in_=io[f"idx_{br}"][:, :])
        idx_sb[br + "r"] = cpool.tile([128, BC // 16], I16, tag=f"rel_{br}",
                                      name=f"rel_{br}")
        nc.sync.dma_start(out=idx_sb[br + "r"][:], in_=io[f"rel_{br}"][:, :])
    w_sb = {}
    for br in ("t", "c"):
        w_sb[br] = cpool.tile([128, 3, NPR], BF16, tag=f"w_{br}",
                              name=f"w_{br}")
        nc.sync.dma_start(out=w_sb[br][:],
                          in_=io[f"w{br}"].rearrange("(j p) c -> p j c", p=128))
    labels = cpool.tile([128, NB], F32, tag="labels")
    nc.sync.dma_start(out=labels[:], in_=io["labels"][:, :])
    times = cpool.tile([1, BC], F32, tag="times")
    nc.sync.dma_start(out=times[:], in_=io["times"][:, :])
    h1k = cpool.tile([H, 1], F32, tag="h1k")
    nc.sync.dma_start(out=h1k[:], in_=io["h1k"][:, :])
    h1b = cpool.tile([H, 1], F32, tag="h1b")
    nc.sync.dma_start(out=h1b[:], in_=io["h1b"][:, :])
    h2kb = cpool.tile([H + 1, H], BF16, tag="h2kb")
    nc.sync.dma_start(out=h2kb[:], in_=io["h2kb"][:, :])
    vr = cpool.tile([H, R], F32, tag="vr")
    nc.sync.dma_start(out=vr[:], in_=io["vr"][:, :])
    ones1 = cpool.tile([1, H], F32, tag="ones1")
    nc.vector.memset(ones1[:], 1.0)

    # ---- gathers -------------------------------------------------------
    # Stage 1 (per branch): 4 sorted-segment row gathers table -> SBUF
    # scratch; scratch row j lands at partition j%128, slot j//128, which
    # is exactly the tokens_per_rank=128 layout the SBUF-source transpose
    # gather expects (token id == scratch row id).
    scratch = {}
    embT = {"t": [], "c": []}

    def seg_gathers(br, tab, q):
        scratch[br] = cpool.tile([128, S_TOT // 128, EPAD], BF16,
                                 tag=f"scr_{br}", name=f"scr_{br}")
        off = 0
        for s in range(4):
            cap = SEG_CAP[s]
            seg_len = min(32768, V - SEG_BASE[s])
            nc.gpsimd.dma_gather(
                scratch[br][:, off // 128:(off + cap) // 128, :],
                tab[SEG_BASE[s]:SEG_BASE[s] + seg_len, :],
                idx_sb[br][:, off // 16:(off + cap) // 16],
                cap, cap, EPAD, queue_num=0, single_packet=False)
            off += cap

    def realign(br, g, ntok):
        e = cpool.tile([128, 3, ntok], BF16, tag=f"embT_{br}{g}",
                       name=f"embT_{br}{g}")
        nc.gpsimd.dma_gather(
            e[:], scratch[br][:],
            idx_sb[br + "r"][:, (ntok // 16) * g:(ntok // 16) * (g + 1)],
            ntok, ntok, EPAD, transpose=True, queue_num=0,
            single_packet=False,
            sbuf_tokens_per_rank=128, sbuf_free_dim_per_rank=EPAD * 2)
        embT[br].append(e)

    # Issue order keeps the single q7 descgen queue busy with no idle
    # holes: target realigns 0-1 let compute-t start early, context
    # segments stream their DMAs while rt2/rt3 descgen runs, and the
    # context realigns find scratch_c already landed.
    seg_gathers("t", io["ttab"], 0)
    realign("t", 0, GTOK)
    realign("t", 1, GTOK)
    seg_gathers("c", io["ctab"], 0)
    realign("t", 2, GTOK)
    realign("t", 3, GTOK)
    for g in range(NG):
        realign("c", g, GTOK)

    # ---- time MLP -> c' coefficients, batched on PE/ACT ----------------
    # h1T[j, b] = tanh(h1k[j] * t[b] + h1b[j]); tv = tanh(h2kb.T @ h1T);
    # c' chunk g = tvT_g.T @ vr, bf16, with homogeneous slot k=R set to 1.
    pmlp = ctx.enter_context(tc.tile_pool(name="pmlp", bufs=2, space="PSUM"))
    pcf = ctx.enter_context(tc.tile_pool(name="pcf", bufs=1, space="PSUM"))
    h1T = cpool.tile([H + 1, BC], BF16, tag="h1T")
    nc.vector.memset(h1T[96:H + 1, :], 1.0)   # row 100 stays 1; 96-99 overwritten
    tvT = cpool.tile([H, BC], F32, tag="tvT")
    HB = BC // 4
    for i in range(4):
        sl = slice(HB * i, HB * (i + 1))
        bc = pmlp.tile([H, HB], F32, tag="mlp", name=f"bcast{i}")
        nc.tensor.matmul(bc[:], ones1[:], times[0:1, sl], start=True,
                         stop=True)
        nc.scalar.activation(h1T[0:H, sl], bc[:], AF.Tanh, bias=h1b[:],
                             scale=h1k[:])
        tvp = pmlp.tile([H, HB], F32, tag="mlp", name=f"tvp{i}")
        nc.tensor.matmul(tvp[:], h2kb[:], h1T[:, sl], start=True, stop=True)
        nc.scalar.activation(tvT[:, sl], tvp[:], AF.Tanh)
    call = cpool.tile([128, NB, RK], BF16, tag="call")
    cfp = pcf.tile([128, NB * R], F32, tag="cfp")
    for g in range(NB):
        nc.tensor.matmul(cfp[:, R * g:R * (g + 1)],
                         tvT[:, 128 * g:128 * (g + 1)], vr[:],
                         start=True, stop=True)
        nc.scalar.copy(call[:, g, 0:R], cfp[:, R * g:R * (g + 1)])
    nc.vector.memset(call[:, :, R:RK], 1.0)

    # ---- main loop: matU = embT.T @ W ; mv = sum_k matU * c' ------------
    mv_all = {}
    junk = cpool.tile([128, NB, MH], F32, tag="junk")
    logits = cpool.tile([128, NB], F32, tag="logits")
    ab = cpool.tile([128, NB], F32, tag="ab")
    ex = cpool.tile([128, NB], F32, tag="ex")
    rl = cpool.tile([128, NB], F32, tag="rl")
    lyj = cpool.tile([128, NB], F32, tag="lyj")
    sacc = cpool.tile([128, NG + 1], F32, tag="sacc")
    for br in ("t", "c"):
        mv_all[br] = cpool.tile([128, NB, MH], F32, tag=f"mv_{br}",
                                name=f"mv_{br}")
        for p2 in range(NB // 2):
            c0 = 2 * p2
            g = c0 // NG
            matU = pmm.tile([128, 2, 512], F32, tag="matU",
                            name=f"mU_{br}{p2}")
            for i in range(2):
                m = (c0 + i) % NG
                for j in range(3):
                    nc.tensor.matmul(
                        matU[:, i, 0:NPR],
                        embT[br][g][:, j, 128 * m:128 * (m + 1)],
                        w_sb[br][:, j, :], start=(j == 0), stop=(j == 2))
            prod = wpool.tile([128, 2, NPR], BF16, tag="prod")
            in0 = matU[:, :, 0:NPR].rearrange("p a (b k) -> p a b k", k=RK)
            in1 = call[:, c0:c0 + 2, 0:RK].rearrange("p (a o) k -> p a o k",
                                                     o=1)
            b0, b1 = bass.broadcast_tensor_aps(in0, in1)
            nc.vector.tensor_tensor(
                prod[:].rearrange("p a (b k) -> p a b k", k=RK), b0, b1,
                op=OP.mult)
            nc.vector.reduce_sum(
                out=mv_all[br][:, c0:c0 + 2, :],
                in_=prod[:].rearrange("p a (b k) -> p a b k", k=RK),
                axis=AX.X)
            c = c0 + 1
            if br == "c" and c % NG == NG - 1:
                # logits + softplus pieces for this group while gathers
                # stream; Abs/Exp/Relu share the tanh act table (no swap),
                # the lone Ln runs batched at the very end.
                g4 = c // NG
                gs = slice(c - NG + 1, c + 1)
                nc.vector.tensor_mul(junk[:, gs, :], mv_all["t"][:, gs, :],
                                     mv_all["c"][:, gs, :])
                nc.vector.reduce_sum(out=logits[:, gs], in_=junk[:, gs, :],
                                     axis=AX.X)
                nc.scalar.activation(ab[:, gs], logits[:, gs], AF.Abs)
                nc.scalar.activation(ex[:, gs], ab[:, gs], AF.Exp, scale=-1.0)
                nc.scalar.activation(rl[:, gs], logits[:, gs], AF.Relu,
                                     accum_out=sacc[:, g4:g4 + 1])
                nc.vector.tensor_mul(lyj[:, gs], logits[:, gs], labels[:, gs])

    # ---- final reduction ------------------------------------------------
    l1p = cpool.tile([128, NB], F32, tag="l1p")
    nc.scalar.activation(l1p[:], ex[:], AF.Ln, bias=1.0,
                         accum_out=sacc[:, NG:NG + 1])
    spos = cpool.tile([128, 1], F32, tag="spos")
    nc.vector.reduce_sum(out=spos[:], in_=sacc[:], axis=AX.X)
    s2 = cpool.tile([128, 1], F32, tag="s2")
    nc.vector.reduce_sum(out=s2[:], in_=lyj[:], axis=AX.X)
    srow = cpool.tile([128, 1], F32, tag="srow")
    nc.vector.tensor_sub(srow[:], spos[:], s2[:])
    nc.sync.dma_start(out=io["out"][:, :], in_=srow[:])


_PROGRAM = None


def _get_program():
    global _PROGRAM
    if _PROGRAM is not None:
        return _PROGRAM
    nc = bacc.Bacc("TRN2", target_bir_lowering=False, debug=False,
                   num_devices=N_CORES)
    io = {
        "ttab": nc.dram_tensor("ttab", [V, EPAD], BF16, kind="ExternalInput").ap(),
        "ctab": nc.dram_tensor("ctab", [V, EPAD], BF16, kind="ExternalInput").ap(),
        "wt": nc.dram_tensor("wt", [EPAD, NPR], BF16, kind="ExternalInput").ap(),
        "wc": nc.dram_tensor("wc", [EPAD, NPR], BF16, kind="ExternalInput").ap(),
        "times": nc.dram_tensor("times", [1, BC], F32, kind="ExternalInput").ap(),
        "h1k": nc.dram_tensor("h1k", [H, 1], F32, kind="ExternalInput").ap(),
        "h1b": nc.dram_tensor("h1b", [H, 1], F32, kind="ExternalInput").ap(),
        "h2kb": nc.dram_tensor("h2kb", [H + 1, H], BF16, kind="ExternalInput").ap(),
        "vr": nc.dram_tensor("vr", [H, R], F32, kind="ExternalInput").ap(),
        "labels": nc.dram_tensor("labels", [128, NB], F32, kind="ExternalInput").ap(),
        "idx_t": nc.dram_tensor("idx_t", [128, S_TOT // 16], I16, kind="ExternalInput").ap(),
        "idx_c": nc.dram_tensor("idx_c", [128, S_TOT // 16], I16, kind="ExternalInput").ap(),
        "rel_t": nc.dram_tensor("rel_t", [128, BC // 16], I16, kind="ExternalInput").ap(),
        "rel_c": nc.dram_tensor("rel_c", [128, BC // 16], I16, kind="ExternalInput").ap(),
        "out": nc.dram_tensor("out", [128, 1], F32, kind="ExternalOutput").ap(),
    }
    with tile.TileContext(nc) as tc:
        with ExitStack() as ctx:
            _build_kernel(ctx, tc, io)
    nc.compile()
    _PROGRAM = nc
    return nc


def _pad_table(tab):
    out = np.zeros((V, EPAD), dtype=ml_dtypes.bfloat16)
    out[:, :EMB] = np.asarray(tab).astype(ml_dtypes.bfloat16)
    out[:, EMB] = 1.0
    return out


def _tv_curve(h1_k, h1_b, h2_k, h2_b, t):
    h1 = np.tanh(t.reshape(-1, 1) @ np.asarray(h1_k, np.float64).reshape(1, H)
                 + np.asarray(h1_b, np.float64).reshape(H))
    return np.tanh(h1 @ np.asarray(h2_k, np.float64)
                   + np.asarray(h2_b, np.float64).reshape(H))


def _tv_basis(h1_k, h1_b, h2_k, h2_b):
    """Top-R right singular basis of the tv curve (weights-only precompute)."""
    g = np.linspace(0.0, 1.0, 2049, dtype=np.float64)
    tvg = _tv_curve(h1_k, h1_b, h2_k, h2_b, g)
    _, _, vt = np.linalg.svd(tvg, full_matrices=False)
    return np.ascontiguousarray(vt[:R].T)          # [100, R]


def build_in_maps(targets, contexts, times, labels, targetemb, contextemb,
                  h1_k, h1_b, h2_k, h2_b, evoke_k, evoke_b, last_k, last_b):
    ttab = _pad_table(targetemb)
    ctab = _pad_table(contextemb)
    vrb = _tv_basis(h1_k, h1_b, h2_k, h2_b)        # [100, R] float64
    h1kc = np.asarray(h1_k, np.float32).reshape(1, H).T.copy()
    h1bc = np.asarray(h1_b, np.float32).reshape(H, 1).copy()
    h2kb = np.vstack([np.asarray(h2_k), np.asarray(h2_b).reshape(1, H)]
                     ).astype(ml_dtypes.bfloat16)

    # Wr[e, p, k] = sum_h evoke_pad[e, p*100+h] * Vr[h, k]
    evoke_pad = np.zeros((EPAD, H * H), dtype=np.float64)
    evoke_pad[:EMB, :] = np.asarray(evoke_k, np.float64)
    evoke_pad[EMB, :] = np.asarray(evoke_b, np.float64)
    wr4 = (evoke_pad.reshape(EPAD * H, H) @ vrb).reshape(EPAD, H, R)

    # target branch: homogeneous slot (p=100, k=R) = 1
    wt = np.zeros((EPAD, MH, RK), dtype=np.float64)
    wt[:, :H, :R] = wr4
    wt[EMB, H, R] = 1.0

    # context branch: fold Gh = last_kh @ last_kh.T
    lastkh = np.vstack([np.asarray(last_k, np.float64),
                        np.asarray(last_b, np.float64).reshape(1, EMB)])
    gh = lastkh @ lastkh.T                         # [101, 101]
    wc = np.zeros((EPAD, MH, RK), dtype=np.float64)
    wc[:, :, :R] = np.einsum("pq,eqk->epk", gh[:, :H], wr4)
    wc[EMB, :, R] = gh[:, H]

    wt = wt.reshape(EPAD, NPR).astype(ml_dtypes.bfloat16)
    wc = wc.reshape(EPAD, NPR).astype(ml_dtypes.bfloat16)

    targets = np.asarray(targets)
    contexts = np.asarray(contexts)
    times = np.asarray(times).astype(np.float32)
    labels = np.asarray(labels).astype(np.float32)

    in_maps = []
    for k in range(N_CORES):
        sl = slice(k * BC, (k + 1) * BC)
        idx_t, rel_t = _prep_indices(targets[sl])
        idx_c, rel_c = _prep_indices(contexts[sl])
        in_maps.append({
            "ttab": ttab, "ctab": ctab, "wt": wt, "wc": wc,
            "times": times[sl].reshape(1, BC),
            "h1k": h1kc, "h1b": h1bc, "h2kb": h2kb,
            "vr": vrb.astype(np.float32),
            "labels": labels[sl].reshape(NB, 128).T.copy(),
            "idx_t": idx_t, "idx_c": idx_c, "rel_t": rel_t, "rel_c": rel_c,
        })
    return in_maps


def kernel(**inputs) -> np.ndarray:
    nc = _get_program()
    in_maps = build_in_maps(**inputs)
    r = run_bass_kernel_spmd(nc, in_maps, list(range(N_CORES)))
    total = np.float64(0.0)
    for m in r.results:
        total += np.asarray(m["out"], np.float64).sum()
    return np.float32(total / B)
